# revision 16
# baseline (speedup 1.0000x reference)
"""Binary-weight dense layer on 8 trn2 NeuronCores.

Computes out[b,s,f] = scale * sum_i x[b,s,i] * (kernel[i,f] ? +1 : -1)
for x [4, 4096, 1024] f32, kernel [1024, 1024] bool, scale scalar f32.

Strategy: data-parallel over the 16384 rows (2048 rows/core).  The
contraction is split into an fp8-e4m3 half (k 0..511) that runs through
the PE in DoubleRow mode (2 contraction planes per matmul, ~1.8x
throughput) and a bf16 half (k 512..1023) at the normal rate.  The
weights are exactly +-scale (scale = 2^-5), representable in both e4m3
and bf16, so the only quantization loss is rounding x's fp8 half
(rel-err 0.0177 on the fixed inputs, under the 2e-2 gate).

Schedule per core: pass 1 covers the first 4 row-tiles chunk-outer with
nh-granular (128 KB) need-ordered input DMAs so the PE starts as soon
as the first two 128 KB pieces land and never outruns the ~358 GB/s
HBM stream.  Remaining 12 row-tiles run tile-sequential (all inputs
resident by then), which staggers the expensive PSUM drains (~1 us per
bank) instead of bunching them behind the last matmul.  Drains split
across DVE (lo half) and ACT (hi half); stores alternate the two HWDGE
rings; the final tile's store is split across both rings.  Outputs are
bf16, upcast on the host.
"""

import numpy as np
import ml_dtypes

import concourse.bacc as bacc
import concourse.mybir as mybir
import concourse.tile as tile
from concourse.bass_utils import run_bass_kernel_spmd

N_CORES = 8
B, S, K, N = 4, 4096, 1024, 1024
ROWS = B * S                     # 16384
RPC = ROWS // N_CORES            # 2048 rows per core
P = 128                          # partitions
G = 4                            # row-groups per core
MG = 4                           # m-tiles per group
GR = MG * P                      # 512 rows per group
KD = 512                         # e4m3 DoubleRow half of the contraction
NH = 512                         # one PSUM bank of f32 (matmul free dim)
MT = G * MG                      # 16 m-tiles per core

_module_cache = {}


def build_module():
    nc = bacc.Bacc(None)
    f8 = mybir.dt.float8e4
    bf = mybir.dt.bfloat16
    f32 = mybir.dt.float32
    DR = mybir.MatmulPerfMode.DoubleRow
    Copy = mybir.ActivationFunctionType.Copy

    # x fp8 half:  xdr[g, kp, ph, r]  = x[g*GR + r, ph*P + kp]    (k = ph*P+kp)
    # x bf16 half: xb[g, kp, j, r]    = x[g*GR + r, KD + j*P + kp]
    # w fp8 half:  wdr[nh, kp, ph, n] = +-scale at [ph*P + kp, nh*NH + n]
    # w bf16 half: wb[nh, kp, j, n]   = +-scale at [KD + j*P + kp, nh*NH + n]
    xdr = nc.dram_tensor("xdr", [G, P, 4, GR], f8, kind="ExternalInput")
    xb = nc.dram_tensor("xb", [G, P, 4, GR], bf, kind="ExternalInput")
    wdr = nc.dram_tensor("wdr", [2, P, 4, NH], f8, kind="ExternalInput")
    wb = nc.dram_tensor("wb", [2, P, 4, NH], bf, kind="ExternalInput")
    out = nc.dram_tensor("out", [RPC, N], bf, kind="ExternalOutput")

    with tile.TileContext(nc) as tc:
        with (
            tc.tile_pool(name="persist", bufs=1) as persist,
            tc.tile_pool(name="psum", bufs=1, space="PSUM") as ps_pool,
            tc.tile_pool(name="outp", bufs=4) as out_pool,
        ):
            # Dummy matmuls bridge the PE-idle window while the first input
            # pieces are in flight so the HAM clock-gate releases (2.4 GHz)
            # soon after the real stream starts.  DVE memset starts early
            # (gpsimd is slow to spin up); uninitialized SBUF would fault.
            wu = persist.tile([P, 384], bf, tag="wu")
            nc.vector.memset(wu, 0)
            warm_ps = ps_pool.tile([P, N], f32, tag="ps0", name="warmps")
            for _ in range(12):
                nc.tensor.matmul(warm_ps[:, 0:256], wu[:, 0:P], wu[:, P:384],
                                 start=True, stop=True)

            # Input DMAs; program order per ring = HWDGE FIFO = need order.
            # Pieces zig-zag across the two rings so the early, latency-
            # critical pieces see both rings' combined HBM bandwidth.
            wdr_t = persist.tile([P, 2, 4, NH], f8, tag="wdr")
            wb_t = persist.tile([P, 2, 4, NH], bf, tag="wb")
            xdr_t = [persist.tile([P, 4, GR], f8, tag=f"xdr{g}",
                                  name=f"xdr{g}") for g in range(G)]
            xb_t = [persist.tile([P, 4, GR], bf, tag=f"xb{g}",
                                 name=f"xb{g}") for g in range(G)]

            def w8(nh, c):   # one [P, 2, NH] piece of the fp8 weights
                return (wdr_t[:, nh, 2 * c:2 * c + 2, :],
                        wdr[nh, :, 2 * c:2 * c + 2, :])

            def wB(nh, jp):  # one [P, 2, NH] piece of the bf16 weights
                return (wb_t[:, nh, 2 * jp:2 * jp + 2, :],
                        wb[nh, :, 2 * jp:2 * jp + 2, :])

            sync_q = [w8(0, 0), w8(1, 0), w8(0, 1), wB(0, 0), wB(0, 1),
                      wB(1, 1),
                      (xb_t[1], xb[1]), (xb_t[2], xb[2])]
            scal_q = [(xdr_t[0][:, 0:2, :], xdr[0][:, 0:2, :]),
                      (xdr_t[0][:, 2:4, :], xdr[0][:, 2:4, :]),
                      w8(1, 1),
                      (xb_t[0][:, 0:2, :], xb[0][:, 0:2, :]),
                      wB(1, 0),
                      (xb_t[0][:, 2:4, :], xb[0][:, 2:4, :]),
                      (xdr_t[1], xdr[1]), (xdr_t[2], xdr[2]),
                      (xdr_t[3], xdr[3]), (xb_t[3], xb[3])]
            for dst, src in sync_q:
                nc.sync.dma_start(out=dst, in_=src)
            for dst, src in scal_q:
                nc.scalar.dma_start(out=dst, in_=src)

            def mm(g, ml, c, nh, ps, start, stop):
                o = ps[:, nh * NH:(nh + 1) * NH]
                if c < 2:
                    lhsT = xdr_t[g][:, 2 * c:2 * c + 2, ml * P:(ml + 1) * P]
                    rhs = wdr_t[:, nh, 2 * c:2 * c + 2, :]
                    nc.tensor.matmul(o, lhsT, rhs, start=start, stop=stop,
                                     perf_mode=DR)
                else:
                    j = c - 2
                    lhsT = xb_t[g][:, j, ml * P:(ml + 1) * P]
                    rhs = wb_t[:, nh, j, :]
                    nc.tensor.matmul(o, lhsT, rhs, start=start, stop=stop)

            def evict(m, ps):
                ot = out_pool.tile([P, N], bf, tag="ot")
                nc.vector.tensor_copy(ot[:, 0:NH], ps[:, 0:NH])
                nc.scalar.activation(ot[:, NH:N], ps[:, NH:N], Copy)
                if m == MT - 1:
                    # split the final store across both rings: tail is one
                    # 128 KB transfer per ring
                    nc.sync.dma_start(out=out[m * P:(m + 1) * P, 0:NH],
                                      in_=ot[:, 0:NH])
                    nc.scalar.dma_start(out=out[m * P:(m + 1) * P, NH:N],
                                        in_=ot[:, NH:N])
                else:
                    ring = nc.sync if m % 2 == 0 else nc.scalar
                    ring.dma_start(out=out[m * P:(m + 1) * P, :], in_=ot)

            # Pass 1: group 0 chunk-outer, nh-outer sweeps (each 4-matmul
            # sweep consumes one 128 KB w piece as it lands).
            pss = [ps_pool.tile([P, N], f32, tag=f"ps{ml}", name=f"psA{ml}")
                   for ml in range(MG)]
            ots = [out_pool.tile([P, N], bf, tag="ot", name=f"otA{ml}")
                   for ml in range(MG)]
            for c in range(6):
                for nh in range(2):
                    for ml in range(MG):
                        mm(0, ml, c, nh, pss[ml],
                           start=(c == 0), stop=(c == 5))
                        # drain each bank as its accumulation finishes so
                        # the first tile-sequential tile can reuse ps0 at
                        # the pass boundary without waiting on 4 drains
                        if c == 5 and nh == 0:
                            nc.vector.tensor_copy(ots[ml][:, 0:NH],
                                                  pss[ml][:, 0:NH])
                        elif c == 5 and nh == 1:
                            nc.scalar.activation(ots[ml][:, NH:N],
                                                 pss[ml][:, NH:N], Copy)
                            ring = nc.sync if ml % 2 == 0 else nc.scalar
                            ring.dma_start(out=out[ml * P:(ml + 1) * P, :],
                                           in_=ots[ml])

            # Passes 2-4: tile-sequential; inputs resident, drains stagger.
            for m in range(MG, MT):
                g, ml = divmod(m, MG)
                ps = ps_pool.tile([P, N], f32, tag=f"ps{m % MG}",
                                  name=f"ps{m}")
                # A tile whose first matmul is DoubleRow with start=True
                # stalls ~0.4 us; rotate so every tile enters on a bf16
                # chunk with the DR chunks mid-tile, and consecutive tiles
                # meet on same-mode boundaries.
                order = [5, 0, 1, 2, 3, 4] if (m - MG) % 2 == 0 else \
                    [4, 3, 2, 1, 0, 5]
                if m < MT - 1:
                    for ci, c in enumerate(order):
                        for nh in range(2):
                            mm(g, ml, c, nh, ps,
                               start=(ci == 0), stop=(ci == 5))
                    evict(m, ps)
                else:
                    # last tile: finish the lo half first and drain it while
                    # the hi half's matmuls run, so the tail is one 128 KB
                    # store per ring
                    ot = out_pool.tile([P, N], bf, tag="ot")
                    for ci, c in enumerate(order):
                        mm(g, ml, c, 0, ps, start=(ci == 0), stop=(ci == 5))
                    nc.vector.tensor_copy(ot[:, 0:NH], ps[:, 0:NH])
                    nc.sync.dma_start(out=out[m * P:(m + 1) * P, 0:NH],
                                      in_=ot[:, 0:NH])
                    for ci, c in enumerate(order):
                        mm(g, ml, c, 1, ps, start=(ci == 0), stop=(ci == 5))
                    nc.vector.tensor_copy(ot[:, NH:N], ps[:, NH:N])
                    nc.scalar.dma_start(out=out[m * P:(m + 1) * P, NH:N],
                                        in_=ot[:, NH:N])
    nc.finalize()
    return nc


def get_module():
    if "nc" not in _module_cache:
        _module_cache["nc"] = build_module()
    return _module_cache["nc"]


def _prepare_in_maps(x, kernel, scale):
    f8 = ml_dtypes.float8_e4m3
    bf16 = ml_dtypes.bfloat16
    x2d = np.asarray(x, dtype=np.float32).reshape(ROWS, K)
    scale = np.float32(scale)
    ws = np.where(np.asarray(kernel, dtype=bool), scale, -scale).astype(np.float32)
    # +-scale is a power of two: exact in e4m3 and bf16
    wdr_h = np.ascontiguousarray(
        ws[:KD].reshape(4, P, 2, NH).transpose(2, 1, 0, 3).astype(f8))
    wb_h = np.ascontiguousarray(
        ws[KD:].reshape(4, P, 2, NH).transpose(2, 1, 0, 3).astype(bf16))
    in_maps = []
    for c in range(N_CORES):
        xc = x2d[c * RPC:(c + 1) * RPC]
        a = xc[:, :KD].reshape(G, GR, 4, P).transpose(0, 3, 2, 1)
        b = xc[:, KD:].reshape(G, GR, 4, P).transpose(0, 3, 2, 1)
        in_maps.append({
            "xdr": np.ascontiguousarray(a.astype(f8)),
            "xb": np.ascontiguousarray(b.astype(bf16)),
            "wdr": wdr_h,
            "wb": wb_h,
        })
    return in_maps


def kernel(x, kernel, scale):
    nc = get_module()
    in_maps = _prepare_in_maps(x, kernel, scale)
    res = run_bass_kernel_spmd(nc, in_maps, core_ids=list(range(N_CORES)))
    out = np.concatenate(
        [np.asarray(r["out"], dtype=np.float32) for r in res.results], axis=0)
    return out.reshape(B, S, N)


# revision 21
# speedup vs baseline: 1.0580x; 1.0580x over previous
"""Binary-weight dense layer on 8 trn2 NeuronCores.

Computes out[b,s,f] = scale * sum_i x[b,s,i] * (kernel[i,f] ? +1 : -1)
for x [4, 4096, 1024] f32, kernel [1024, 1024] bool, scale scalar f32.

Strategy: data-parallel over the 16384 rows (2048 rows/core).  The
contraction is split into an fp8-e4m3 half (k 0..511) that runs through
the PE in DoubleRow mode (2 contraction planes per matmul, ~1.8x
throughput) and a bf16 half (k 512..1023) at the normal rate.  The
weights are exactly +-scale (scale = 2^-5), representable in both e4m3
and bf16, so the only quantization loss is rounding x's fp8 half
(rel-err 0.0177 on the fixed inputs, under the 2e-2 gate).

Schedule per core: pass 1 covers the first 4 row-tiles chunk-outer with
nh-granular (128 KB) need-ordered input DMAs so the PE starts as soon
as the first two 128 KB pieces land and never outruns the ~358 GB/s
HBM stream.  Remaining 12 row-tiles run tile-sequential (all inputs
resident by then), which staggers the expensive PSUM drains (~1 us per
bank) instead of bunching them behind the last matmul.  Drains split
across DVE (lo half) and ACT (hi half); stores alternate the two HWDGE
rings; the final tile's store is split across both rings.  Outputs are
bf16, upcast on the host.
"""

import numpy as np
import ml_dtypes

import concourse.bacc as bacc
import concourse.mybir as mybir
import concourse.tile as tile
from concourse.bass_utils import run_bass_kernel_spmd

N_CORES = 8
B, S, K, N = 4, 4096, 1024, 1024
ROWS = B * S                     # 16384
RPC = ROWS // N_CORES            # 2048 rows per core
P = 128                          # partitions
G = 4                            # row-groups per core
MG = 4                           # m-tiles per group
GR = MG * P                      # 512 rows per group
KD = 512                         # e4m3 DoubleRow half of the contraction
NH = 512                         # one PSUM bank of f32 (matmul free dim)
MT = G * MG                      # 16 m-tiles per core

_module_cache = {}


def build_module():
    nc = bacc.Bacc(None)
    f8 = mybir.dt.float8e4
    bf = mybir.dt.bfloat16
    f32 = mybir.dt.float32
    DR = mybir.MatmulPerfMode.DoubleRow
    Copy = mybir.ActivationFunctionType.Copy

    # x fp8 half:  xdr[g, kp, ph, r]  = x[g*GR + r, ph*P + kp]    (k = ph*P+kp)
    # x bf16 half: xb[g, kp, j, r]    = x[g*GR + r, KD + j*P + kp]
    # w fp8 half:  wdr[nh, kp, ph, n] = +-scale at [ph*P + kp, nh*NH + n]
    # w bf16 half: wb[nh, kp, j, n]   = +-scale at [KD + j*P + kp, nh*NH + n]
    xdr = nc.dram_tensor("xdr", [G, P, 4, GR], f8, kind="ExternalInput")
    xb = nc.dram_tensor("xb", [G, P, 4, GR], bf, kind="ExternalInput")
    wdr = nc.dram_tensor("wdr", [2, P, 4, NH], f8, kind="ExternalInput")
    wb = nc.dram_tensor("wb", [2, P, 4, NH], bf, kind="ExternalInput")
    out = nc.dram_tensor("out", [RPC, N], bf, kind="ExternalOutput")

    with tile.TileContext(nc) as tc:
        with (
            tc.tile_pool(name="persist", bufs=1) as persist,
            tc.tile_pool(name="psum", bufs=1, space="PSUM") as ps_pool,
            tc.tile_pool(name="outp", bufs=4) as out_pool,
        ):
            # Dummy matmuls bridge the PE-idle window while the first input
            # pieces are in flight so the HAM clock-gate releases (2.4 GHz)
            # soon after the real stream starts.  DVE memset starts early
            # (gpsimd is slow to spin up); uninitialized SBUF would fault.
            wu = persist.tile([P, 384], bf, tag="wu")
            nc.vector.memset(wu, 0)
            warm_ps = ps_pool.tile([P, NH], f32, tag="psa0", name="warmps")
            for _ in range(12):
                nc.tensor.matmul(warm_ps[:, 0:256], wu[:, 0:P], wu[:, P:384],
                                 start=True, stop=True)

            # Input DMAs; program order per ring = HWDGE FIFO = need order.
            # Pieces zig-zag across the two rings so the early, latency-
            # critical pieces see both rings' combined HBM bandwidth.
            wdr_t = persist.tile([P, 2, 4, NH], f8, tag="wdr")
            wb_t = persist.tile([P, 2, 4, NH], bf, tag="wb")
            xdr_t = [persist.tile([P, 4, GR], f8, tag=f"xdr{g}",
                                  name=f"xdr{g}") for g in range(G)]
            xb_t = [persist.tile([P, 4, GR], bf, tag=f"xb{g}",
                                 name=f"xb{g}") for g in range(G)]

            def w8(nh, c):   # one [P, 2, NH] piece of the fp8 weights
                return (wdr_t[:, nh, 2 * c:2 * c + 2, :],
                        wdr[nh, :, 2 * c:2 * c + 2, :])

            def wB(nh, jp):  # one [P, 2, NH] piece of the bf16 weights
                return (wb_t[:, nh, 2 * jp:2 * jp + 2, :],
                        wb[nh, :, 2 * jp:2 * jp + 2, :])

            sync_q = [w8(0, 0), w8(1, 0), w8(0, 1), wB(0, 0), wB(0, 1),
                      wB(1, 1),
                      (xb_t[1], xb[1]), (xb_t[2], xb[2])]
            scal_q = [(xdr_t[0][:, 0:2, :], xdr[0][:, 0:2, :]),
                      (xdr_t[0][:, 2:4, :], xdr[0][:, 2:4, :]),
                      w8(1, 1),
                      (xb_t[0][:, 0:2, :], xb[0][:, 0:2, :]),
                      wB(1, 0),
                      (xb_t[0][:, 2:4, :], xb[0][:, 2:4, :]),
                      (xdr_t[1], xdr[1]), (xdr_t[2], xdr[2]),
                      (xdr_t[3], xdr[3]), (xb_t[3], xb[3])]
            for dst, src in sync_q:
                nc.sync.dma_start(out=dst, in_=src)
            for dst, src in scal_q:
                nc.scalar.dma_start(out=dst, in_=src)

            # PSUM tiles are one bank per nh half (two tiles per row-tile):
            # Tile tracks PSUM hazards per tile, so a split lets one half
            # drain while the other half still accumulates.
            def mm(g, ml, c, nh, psp, start, stop):
                o = psp[nh][:, :]
                if c < 2:
                    lhsT = xdr_t[g][:, 2 * c:2 * c + 2, ml * P:(ml + 1) * P]
                    rhs = wdr_t[:, nh, 2 * c:2 * c + 2, :]
                    nc.tensor.matmul(o, lhsT, rhs, start=start, stop=stop,
                                     perf_mode=DR)
                else:
                    j = c - 2
                    lhsT = xb_t[g][:, j, ml * P:(ml + 1) * P]
                    rhs = wb_t[:, nh, j, :]
                    nc.tensor.matmul(o, lhsT, rhs, start=start, stop=stop)

            def evict(m, psp):
                ot = out_pool.tile([P, N], bf, tag="ot")
                nc.vector.tensor_copy(ot[:, 0:NH], psp[0][:, :])
                nc.scalar.activation(ot[:, NH:N], psp[1][:, :], Copy)
                if m == MT - 1:
                    # split the final store across both rings: tail is one
                    # 128 KB transfer per ring
                    nc.sync.dma_start(out=out[m * P:(m + 1) * P, 0:NH],
                                      in_=ot[:, 0:NH])
                    nc.scalar.dma_start(out=out[m * P:(m + 1) * P, NH:N],
                                        in_=ot[:, NH:N])
                else:
                    ring = nc.sync if m % 2 == 0 else nc.scalar
                    ring.dma_start(out=out[m * P:(m + 1) * P, :], in_=ot)

            # Pass 1: group 0 chunk-outer, nh-outer sweeps (each 4-matmul
            # sweep consumes one 128 KB w piece as it lands).
            pss = [(ps_pool.tile([P, NH], f32, tag=f"psa{ml}",
                                 name=f"psAa{ml}"),
                    ps_pool.tile([P, NH], f32, tag=f"psb{ml}",
                                 name=f"psAb{ml}")) for ml in range(MG)]
            ots = [out_pool.tile([P, N], bf, tag="ot", name=f"otA{ml}")
                   for ml in range(MG)]
            for c in range(6):
                for nh in range(2):
                    for ml in range(MG):
                        mm(0, ml, c, nh, pss[ml],
                           start=(c == 0), stop=(c == 5))
                        # drain each bank as its accumulation finishes so
                        # the first tile-sequential tile can reuse psa0 at
                        # the pass boundary without waiting on 4 drains
                        if c == 5 and nh == 0:
                            nc.vector.tensor_copy(ots[ml][:, 0:NH],
                                                  pss[ml][0][:, :])
                        elif c == 5 and nh == 1:
                            nc.scalar.activation(ots[ml][:, NH:N],
                                                 pss[ml][1][:, :], Copy)
                            ring = nc.sync if ml % 2 == 0 else nc.scalar
                            ring.dma_start(out=out[ml * P:(ml + 1) * P, :],
                                           in_=ots[ml])

            # Passes 2-4: tile-sequential; inputs resident, drains stagger.
            for m in range(MG, MT):
                g, ml = divmod(m, MG)
                psp = (ps_pool.tile([P, NH], f32, tag=f"psa{m % MG}",
                                    name=f"psa{m}"),
                       ps_pool.tile([P, NH], f32, tag=f"psb{m % MG}",
                                    name=f"psb{m}"))
                # Alternate chunk order per tile: a bf16->DoubleRow entry
                # stalls ~0.4 us, so merge neighboring tiles' DR sections
                # at their shared boundary (one DR entry per tile pair).
                order = [5, 4, 3, 2, 1, 0] if (m - MG) % 2 == 0 else \
                    [0, 1, 2, 3, 4, 5]
                if m < MT - 1:
                    for ci, c in enumerate(order):
                        for nh in range(2):
                            mm(g, ml, c, nh, psp,
                               start=(ci == 0), stop=(ci == 5))
                    evict(m, psp)
                else:
                    # last tile: finish the lo half first and drain it while
                    # the hi half's matmuls run, so the tail is one 128 KB
                    # store per ring
                    ot = out_pool.tile([P, N], bf, tag="ot")
                    for ci, c in enumerate(order):
                        mm(g, ml, c, 0, psp, start=(ci == 0), stop=(ci == 5))
                    nc.vector.tensor_copy(ot[:, 0:NH], psp[0][:, :])
                    nc.sync.dma_start(out=out[m * P:(m + 1) * P, 0:NH],
                                      in_=ot[:, 0:NH])
                    for ci, c in enumerate(reversed(order)):
                        mm(g, ml, c, 1, psp, start=(ci == 0), stop=(ci == 5))
                    nc.vector.tensor_copy(ot[:, NH:N], psp[1][:, :])
                    nc.scalar.dma_start(out=out[m * P:(m + 1) * P, NH:N],
                                        in_=ot[:, NH:N])
    nc.finalize()
    return nc


def get_module():
    if "nc" not in _module_cache:
        _module_cache["nc"] = build_module()
    return _module_cache["nc"]


def _prepare_in_maps(x, kernel, scale):
    f8 = ml_dtypes.float8_e4m3
    bf16 = ml_dtypes.bfloat16
    x2d = np.asarray(x, dtype=np.float32).reshape(ROWS, K)
    scale = np.float32(scale)
    ws = np.where(np.asarray(kernel, dtype=bool), scale, -scale).astype(np.float32)
    # +-scale is a power of two: exact in e4m3 and bf16
    wdr_h = np.ascontiguousarray(
        ws[:KD].reshape(4, P, 2, NH).transpose(2, 1, 0, 3).astype(f8))
    wb_h = np.ascontiguousarray(
        ws[KD:].reshape(4, P, 2, NH).transpose(2, 1, 0, 3).astype(bf16))
    in_maps = []
    for c in range(N_CORES):
        xc = x2d[c * RPC:(c + 1) * RPC]
        a = xc[:, :KD].reshape(G, GR, 4, P).transpose(0, 3, 2, 1)
        b = xc[:, KD:].reshape(G, GR, 4, P).transpose(0, 3, 2, 1)
        in_maps.append({
            "xdr": np.ascontiguousarray(a.astype(f8)),
            "xb": np.ascontiguousarray(b.astype(bf16)),
            "wdr": wdr_h,
            "wb": wb_h,
        })
    return in_maps


def kernel(x, kernel, scale):
    nc = get_module()
    in_maps = _prepare_in_maps(x, kernel, scale)
    res = run_bass_kernel_spmd(nc, in_maps, core_ids=list(range(N_CORES)))
    out = np.concatenate(
        [np.asarray(r["out"], dtype=np.float32) for r in res.results], axis=0)
    return out.reshape(B, S, N)


# revision 22
# speedup vs baseline: 1.0720x; 1.0132x over previous
"""Binary-weight dense layer on 8 trn2 NeuronCores.

Computes out[b,s,f] = scale * sum_i x[b,s,i] * (kernel[i,f] ? +1 : -1)
for x [4, 4096, 1024] f32, kernel [1024, 1024] bool, scale scalar f32.

Strategy: data-parallel over the 16384 rows (2048 rows/core).  The
contraction is split into an fp8-e4m3 half (k 0..511) that runs through
the PE in DoubleRow mode (2 contraction planes per matmul, ~1.8x
throughput) and a bf16 half (k 512..1023) at the normal rate.  The
weights are exactly +-scale (scale = 2^-5), representable in both e4m3
and bf16, so the only quantization loss is rounding x's fp8 half
(rel-err 0.0177 on the fixed inputs, under the 2e-2 gate).

Schedule per core: pass 1 covers the first 4 row-tiles chunk-outer with
nh-granular (128 KB) need-ordered input DMAs so the PE starts as soon
as the first two 128 KB pieces land and never outruns the ~358 GB/s
HBM stream.  Remaining 12 row-tiles run tile-sequential (all inputs
resident by then), which staggers the expensive PSUM drains (~1 us per
bank) instead of bunching them behind the last matmul.  Drains split
across DVE (lo half) and ACT (hi half); stores alternate the two HWDGE
rings; the final tile's store is split across both rings.  Outputs are
bf16, upcast on the host.
"""

import numpy as np
import ml_dtypes

import concourse.bacc as bacc
import concourse.mybir as mybir
import concourse.tile as tile
from concourse.bass_utils import run_bass_kernel_spmd

N_CORES = 8
B, S, K, N = 4, 4096, 1024, 1024
ROWS = B * S                     # 16384
RPC = ROWS // N_CORES            # 2048 rows per core
P = 128                          # partitions
G = 4                            # row-groups per core
MG = 4                           # m-tiles per group
GR = MG * P                      # 512 rows per group
KD = 512                         # e4m3 DoubleRow half of the contraction
NH = 512                         # one PSUM bank of f32 (matmul free dim)
MT = G * MG                      # 16 m-tiles per core

_module_cache = {}


def build_module():
    nc = bacc.Bacc(None)
    f8 = mybir.dt.float8e4
    bf = mybir.dt.bfloat16
    f32 = mybir.dt.float32
    DR = mybir.MatmulPerfMode.DoubleRow
    Copy = mybir.ActivationFunctionType.Copy

    # x fp8 half:  xdr[g, kp, ph, r]  = x[g*GR + r, ph*P + kp]    (k = ph*P+kp)
    # x bf16 half: xb[g, kp, j, r]    = x[g*GR + r, KD + j*P + kp]
    # w fp8 half:  wdr[nh, kp, ph, n] = +-scale at [ph*P + kp, nh*NH + n]
    # w bf16 half: wb[nh, kp, j, n]   = +-scale at [KD + j*P + kp, nh*NH + n]
    xdr = nc.dram_tensor("xdr", [G, P, 4, GR], f8, kind="ExternalInput")
    xb = nc.dram_tensor("xb", [G, P, 4, GR], bf, kind="ExternalInput")
    wdr = nc.dram_tensor("wdr", [2, P, 4, NH], f8, kind="ExternalInput")
    wb = nc.dram_tensor("wb", [2, P, 4, NH], bf, kind="ExternalInput")
    out = nc.dram_tensor("out", [RPC, N], bf, kind="ExternalOutput")

    with tile.TileContext(nc) as tc:
        with (
            tc.tile_pool(name="persist", bufs=1) as persist,
            tc.tile_pool(name="psum", bufs=1, space="PSUM") as ps_pool,
            tc.tile_pool(name="outp", bufs=4) as out_pool,
        ):
            # Dummy matmuls bridge the PE-idle window while the first input
            # pieces are in flight so the HAM clock-gate releases (2.4 GHz)
            # soon after the real stream starts.  DVE memset starts early
            # (gpsimd is slow to spin up); uninitialized SBUF would fault.
            wu = persist.tile([P, 384], bf, tag="wu")
            nc.vector.memset(wu, 0)
            warm_ps = ps_pool.tile([P, NH], f32, tag="psa0", name="warmps")
            for _ in range(12):
                nc.tensor.matmul(warm_ps[:, 0:256], wu[:, 0:P], wu[:, P:384],
                                 start=True, stop=True)

            # Input DMAs; program order per ring = HWDGE FIFO = need order.
            # Pieces zig-zag across the two rings so the early, latency-
            # critical pieces see both rings' combined HBM bandwidth.
            wdr_t = persist.tile([P, 2, 4, NH], f8, tag="wdr")
            wb_t = persist.tile([P, 2, 4, NH], bf, tag="wb")
            xdr_t = [persist.tile([P, 4, GR], f8, tag=f"xdr{g}",
                                  name=f"xdr{g}") for g in range(G)]
            xb_t = [persist.tile([P, 4, GR], bf, tag=f"xb{g}",
                                 name=f"xb{g}") for g in range(G)]

            def w8(nh, c):   # one [P, 2, NH] piece of the fp8 weights
                return (wdr_t[:, nh, 2 * c:2 * c + 2, :],
                        wdr[nh, :, 2 * c:2 * c + 2, :])

            def wB(nh, jp):  # one [P, 2, NH] piece of the bf16 weights
                return (wb_t[:, nh, 2 * jp:2 * jp + 2, :],
                        wb[nh, :, 2 * jp:2 * jp + 2, :])

            sync_q = [w8(0, 0), w8(1, 0), w8(0, 1), wB(0, 0), wB(0, 1),
                      wB(1, 1),
                      (xb_t[1], xb[1]), (xb_t[2], xb[2])]
            scal_q = [(xdr_t[0][:, 0:2, :], xdr[0][:, 0:2, :]),
                      (xdr_t[0][:, 2:4, :], xdr[0][:, 2:4, :]),
                      w8(1, 1),
                      (xb_t[0][:, 0:2, :], xb[0][:, 0:2, :]),
                      wB(1, 0),
                      (xb_t[0][:, 2:4, :], xb[0][:, 2:4, :]),
                      (xdr_t[1], xdr[1]), (xdr_t[2], xdr[2]),
                      (xdr_t[3], xdr[3]), (xb_t[3], xb[3])]
            for dst, src in sync_q:
                nc.sync.dma_start(out=dst, in_=src)
            for dst, src in scal_q:
                nc.scalar.dma_start(out=dst, in_=src)

            # PSUM tiles are one bank per nh half (two tiles per row-tile):
            # Tile tracks PSUM hazards per tile, so a split lets one half
            # drain while the other half still accumulates.
            def mm(g, ml, c, nh, psp, start, stop):
                o = psp[nh][:, :]
                if c < 2:
                    lhsT = xdr_t[g][:, 2 * c:2 * c + 2, ml * P:(ml + 1) * P]
                    rhs = wdr_t[:, nh, 2 * c:2 * c + 2, :]
                    nc.tensor.matmul(o, lhsT, rhs, start=start, stop=stop,
                                     perf_mode=DR)
                else:
                    j = c - 2
                    lhsT = xb_t[g][:, j, ml * P:(ml + 1) * P]
                    rhs = wb_t[:, nh, j, :]
                    nc.tensor.matmul(o, lhsT, rhs, start=start, stop=stop)

            def evict(m, psp):
                ot = out_pool.tile([P, N], bf, tag="ot")
                nc.vector.tensor_copy(ot[:, 0:NH], psp[0][:, :])
                nc.scalar.activation(ot[:, NH:N], psp[1][:, :], Copy)
                if m == MT - 1:
                    # split the final store across both rings: tail is one
                    # 128 KB transfer per ring
                    nc.sync.dma_start(out=out[m * P:(m + 1) * P, 0:NH],
                                      in_=ot[:, 0:NH])
                    nc.scalar.dma_start(out=out[m * P:(m + 1) * P, NH:N],
                                        in_=ot[:, NH:N])
                else:
                    ring = nc.sync if m % 2 == 0 else nc.scalar
                    ring.dma_start(out=out[m * P:(m + 1) * P, :], in_=ot)

            # Pass 1: group 0 chunk-outer, nh-outer sweeps (each 4-matmul
            # sweep consumes one 128 KB w piece as it lands).
            pss = [(ps_pool.tile([P, NH], f32, tag=f"psa{ml}",
                                 name=f"psAa{ml}"),
                    ps_pool.tile([P, NH], f32, tag=f"psb{ml}",
                                 name=f"psAb{ml}")) for ml in range(MG)]
            ots = [out_pool.tile([P, N], bf, tag="ot", name=f"otA{ml}")
                   for ml in range(MG)]
            for c in range(6):
                for nh in range(2):
                    for ml in range(MG):
                        mm(0, ml, c, nh, pss[ml],
                           start=(c == 0), stop=(c == 5))
                        # drain each bank as its accumulation finishes so
                        # the first tile-sequential tile can reuse psa0 at
                        # the pass boundary without waiting on 4 drains
                        if c == 5 and nh == 0:
                            nc.vector.tensor_copy(ots[ml][:, 0:NH],
                                                  pss[ml][0][:, :])
                        elif c == 5 and nh == 1:
                            nc.scalar.activation(ots[ml][:, NH:N],
                                                 pss[ml][1][:, :], Copy)
                            ring = nc.sync if ml % 2 == 0 else nc.scalar
                            ring.dma_start(out=out[ml * P:(ml + 1) * P, :],
                                           in_=ots[ml])

            # Passes 2-4: blocks of 4 tiles (a,b,c,d).  A bf16->DoubleRow
            # entry stalls ~0.4 us, so each block runs [B(a) B(b) DR(a)
            # DR(b) DR(c) DR(d) B(c) B(d)]: one DR entry per block, all
            # other boundaries same-mode.  a,b drain after their DR
            # chunks, c,d after their B chunks; 8 PSUM banks live at the
            # block midpoint.
            def ps_pair(m):
                return (ps_pool.tile([P, NH], f32, tag=f"psa{m % MG}",
                                     name=f"psa{m}"),
                        ps_pool.tile([P, NH], f32, tag=f"psb{m % MG}",
                                     name=f"psb{m}"))

            for blk in range(3):
                t = [MG + 4 * blk + i for i in range(4)]
                pp = {}
                for m in t[:2]:
                    pp[m] = ps_pair(m)
                    g, ml = divmod(m, MG)
                    for ci, c in enumerate((2, 3, 4, 5)):
                        for nh in range(2):
                            mm(g, ml, c, nh, pp[m],
                               start=(ci == 0), stop=False)
                for m in t[:2]:
                    g, ml = divmod(m, MG)
                    for c in (0, 1):
                        for nh in range(2):
                            mm(g, ml, c, nh, pp[m],
                               start=False, stop=(c == 1))
                    evict(m, pp[m])
                for m in t[2:]:
                    pp[m] = ps_pair(m)
                    g, ml = divmod(m, MG)
                    for c in (0, 1):
                        for nh in range(2):
                            mm(g, ml, c, nh, pp[m],
                               start=(c == 0), stop=False)
                for m in t[2:]:
                    g, ml = divmod(m, MG)
                    if m < MT - 1:
                        for ci, c in enumerate((2, 3, 4, 5)):
                            for nh in range(2):
                                mm(g, ml, c, nh, pp[m],
                                   start=False, stop=(ci == 3))
                        evict(m, pp[m])
                    else:
                        # last tile: finish the lo half first and drain it
                        # while the hi half's matmuls run, so the tail is
                        # one 128 KB store per ring
                        ot = out_pool.tile([P, N], bf, tag="ot")
                        for ci, c in enumerate((2, 3, 4, 5)):
                            mm(g, ml, c, 0, pp[m],
                               start=False, stop=(ci == 3))
                        nc.vector.tensor_copy(ot[:, 0:NH], pp[m][0][:, :])
                        nc.sync.dma_start(out=out[m * P:(m + 1) * P, 0:NH],
                                          in_=ot[:, 0:NH])
                        for ci, c in enumerate((5, 4, 3, 2)):
                            mm(g, ml, c, 1, pp[m],
                               start=False, stop=(ci == 3))
                        nc.vector.tensor_copy(ot[:, NH:N], pp[m][1][:, :])
                        nc.scalar.dma_start(out=out[m * P:(m + 1) * P, NH:N],
                                            in_=ot[:, NH:N])
    nc.finalize()
    return nc


def get_module():
    if "nc" not in _module_cache:
        _module_cache["nc"] = build_module()
    return _module_cache["nc"]


def _prepare_in_maps(x, kernel, scale):
    f8 = ml_dtypes.float8_e4m3
    bf16 = ml_dtypes.bfloat16
    x2d = np.asarray(x, dtype=np.float32).reshape(ROWS, K)
    scale = np.float32(scale)
    ws = np.where(np.asarray(kernel, dtype=bool), scale, -scale).astype(np.float32)
    # +-scale is a power of two: exact in e4m3 and bf16
    wdr_h = np.ascontiguousarray(
        ws[:KD].reshape(4, P, 2, NH).transpose(2, 1, 0, 3).astype(f8))
    wb_h = np.ascontiguousarray(
        ws[KD:].reshape(4, P, 2, NH).transpose(2, 1, 0, 3).astype(bf16))
    in_maps = []
    for c in range(N_CORES):
        xc = x2d[c * RPC:(c + 1) * RPC]
        a = xc[:, :KD].reshape(G, GR, 4, P).transpose(0, 3, 2, 1)
        b = xc[:, KD:].reshape(G, GR, 4, P).transpose(0, 3, 2, 1)
        in_maps.append({
            "xdr": np.ascontiguousarray(a.astype(f8)),
            "xb": np.ascontiguousarray(b.astype(bf16)),
            "wdr": wdr_h,
            "wb": wb_h,
        })
    return in_maps


def kernel(x, kernel, scale):
    nc = get_module()
    in_maps = _prepare_in_maps(x, kernel, scale)
    res = run_bass_kernel_spmd(nc, in_maps, core_ids=list(range(N_CORES)))
    out = np.concatenate(
        [np.asarray(r["out"], dtype=np.float32) for r in res.results], axis=0)
    return out.reshape(B, S, N)
